# revision 2
# baseline (speedup 1.0000x reference)
"""Trainium2 Bass kernel for nn_BatchDelayProcessor.

Computes, per batch row (B=64, T=441000, D=22050 delay, 20 blocks):
    delayed[t] = 0                          , t < D
    delayed[t] = x[t-D] + 0.3*delayed[t-D]  , t >= D
    out[t]     = 0.5*x[t] + 0.5*delayed[t]

Block recurrence with u_k = 0.5*x_k (the 0.5 is folded into the host-side
bf16 conversion -- an exact exponent shift, so u carries bit-identical
information to bf16(x)):
    out_0     = u_0
    out_{k+1} = 0.3*out_k + w_{k+1},   w_{k+1} = 0.7*u_k + u_{k+1}
which splits the 2-ops-per-block recurrence into an off-chain STT (w) and
a 19-op serial chain (out), so dependent DVE ops are never adjacent and
write-drain latency hides under the interleaved w ops.

Perf model (per core, vs the 128us f32 baseline):
  - bf16 I/O halves HBM traffic to 7.06 MB each way; tol is 2e-2 and the
    measured bf16 error is ~6e-3.
  - Host re-layout to (120 partitions x 29400) makes each partition's 20
    blocks DRAM-contiguous: 120 big descriptors per DMA, not 15*8 small.
  - The old kernel's HWDGE loads only ever landed on SDMA engines 0-7
    (trace: Q_X/Q_I slices on 8 engines at ~26 GB/s line rate = the real
    bottleneck).  All DMA here goes through SWDGE (gpsimd), which spreads
    descriptors across all 16 engines; 16 x ~26 GB/s > the 358 GB/s
    per-core HBM limit, so the floor is ~14.1MB/358GB/s ~= 39us.
  - Whole input + output SBUF-resident (3 x 58.8 KB/partition < 208 KB):
    no ring reuse, no WAR tracking.

Sharding: data-parallel over batch -- 8 rows per NeuronCore, no comms.
Per-core layout: partition p = r*15 + s holds row r, sub-slice s; its
29400 columns are [block k][f] with f in [0,1470).
"""

from contextlib import ExitStack

import ml_dtypes
import numpy as np

import concourse.bass as bass
import concourse.mybir as mybir
from concourse.bass_utils import run_bass_kernel_spmd

B, T = 64, 441000
D, NBLK = 22050, 20
NCORES = 8
ROWS = B // NCORES          # 8 rows per core
SPLITS = 15                 # 22050 = 15 * 1470
FREE = D // SPLITS          # 1470 elems per (partition, block)
P = ROWS * SPLITS           # 120 partitions
PW = NBLK * FREE            # 29400 elems per partition

GL = 2                      # blocks per load DMA
NGL = NBLK // GL            # 10 load groups
GS = 2                      # blocks per store DMA
NGS = NBLK // GS            # 10 store groups

BF16 = mybir.dt.bfloat16
BF16_NP = ml_dtypes.bfloat16


def build_nc() -> bass.Bass:
    nc = bass.Bass(trn_type="TRN2")
    x = nc.declare_dram_parameter("x", [P, PW], BF16, isOutput=False)
    y = nc.declare_dram_parameter("y", [P, PW], BF16, isOutput=True)
    xv = x.rearrange("p (g f) -> g p f", g=NGL)
    yv = y.rearrange("p (s f) -> s p f", s=NGS)

    with ExitStack() as ctx:
        block = ctx.enter_context(nc.Block())
        xbuf = ctx.enter_context(nc.sbuf_tensor("xbuf", [P, PW], BF16))
        wbuf = ctx.enter_context(nc.sbuf_tensor("wbuf", [P, PW], BF16))
        obuf = ctx.enter_context(nc.sbuf_tensor("obuf", [P, PW], BF16))
        s_in = [
            ctx.enter_context(nc.semaphore(f"s_in{g}")) for g in range(NGL)
        ]
        s_dve = ctx.enter_context(nc.semaphore("s_dve"))
        s_st = ctx.enter_context(nc.semaphore("s_st"))

        def blk(buf, k, n=1):
            return buf[:, k * FREE : (k + n) * FREE]

        # All DMA on the gpsimd SWDGE path: loads first (no waits -- ring
        # backpressure is the only limiter), then stores gated on compute.
        # s_dve counting: DVE op at position i bumps s_dve to i+1.  Op
        # order is [w1, c0, w2, c1, ..., w19, c18, c19]; c_k sits at
        # position 2k+1, so out_k is drained once s_dve >= 2k+2.
        @block.gpsimd
        def _(gpsimd):
            for g in range(NGL):
                gpsimd.dma_start(
                    out=blk(xbuf, g * GL, GL), in_=xv[g]
                ).then_inc(s_in[g], 16)
            for s in range(NGS):
                k_last = s * GS + GS - 1  # last block in this store group
                gpsimd.wait_ge(s_dve, min(2 * k_last + 2, 2 * NBLK - 1))
                gpsimd.dma_start(
                    out=yv[s], in_=blk(obuf, s * GS, GS)
                ).then_inc(s_st, 16)

        # DVE: interleave off-chain w ops with the serial out chain so a
        # chain op's producer always has a full op's latency to drain.
        @block.vector
        def _(vector):
            for k in range(NBLK - 1):
                # w_{k+1} = (u_k * 0.7) + u_{k+1}   [off-chain]
                if k == 0:
                    vector.wait_ge(s_in[0], 16)
                elif k % 2 == 1:
                    vector.wait_ge(s_in[(k + 1) // 2], 16)
                nc.vector.scalar_tensor_tensor(
                    out=blk(wbuf, k + 1),
                    in0=blk(xbuf, k),
                    scalar=0.7,
                    in1=blk(xbuf, k + 1),
                    op0=mybir.AluOpType.mult,
                    op1=mybir.AluOpType.add,
                ).then_inc(s_dve, 1)
                if k == 0:
                    # out_0 = u_0
                    nc.vector.tensor_copy(blk(obuf, 0), blk(xbuf, 0)).then_inc(
                        s_dve, 1
                    )
                else:
                    # out_k = (out_{k-1} * 0.3) + w_k   [chain]
                    vector.wait_ge(s_dve, 2 * k)
                    nc.vector.scalar_tensor_tensor(
                        out=blk(obuf, k),
                        in0=blk(obuf, k - 1),
                        scalar=0.3,
                        in1=blk(wbuf, k),
                        op0=mybir.AluOpType.mult,
                        op1=mybir.AluOpType.add,
                    ).then_inc(s_dve, 1)
            k = NBLK - 1
            vector.wait_ge(s_dve, 2 * k)
            nc.vector.scalar_tensor_tensor(
                out=blk(obuf, k),
                in0=blk(obuf, k - 1),
                scalar=0.3,
                in1=blk(wbuf, k),
                op0=mybir.AluOpType.mult,
                op1=mybir.AluOpType.add,
            ).then_inc(s_dve, 1)

    return nc


_NC_CACHE = None


def _get_nc() -> bass.Bass:
    global _NC_CACHE
    if _NC_CACHE is None:
        _NC_CACHE = build_nc()
    return _NC_CACHE


def _shard(x: np.ndarray) -> list[dict[str, np.ndarray]]:
    x = np.asarray(x, dtype=np.float32)
    assert x.shape == (B, T), x.shape
    maps = []
    for i in range(NCORES):
        u = (x[i * ROWS : (i + 1) * ROWS] * np.float32(0.5)).reshape(
            ROWS, NBLK, SPLITS, FREE
        )
        u = np.ascontiguousarray(u.transpose(0, 2, 1, 3)).reshape(P, PW)
        maps.append({"x": u.astype(BF16_NP)})
    return maps


def _unshard(results: list[dict[str, np.ndarray]]) -> np.ndarray:
    outs = []
    for r in results:
        yc = np.asarray(r["y"]).astype(np.float32).reshape(
            ROWS, SPLITS, NBLK, FREE
        )
        outs.append(
            np.ascontiguousarray(yc.transpose(0, 2, 1, 3)).reshape(ROWS, T)
        )
    return np.concatenate(outs, axis=0)


def kernel(x: np.ndarray) -> np.ndarray:
    nc = _get_nc()
    res = run_bass_kernel_spmd(nc, _shard(x), core_ids=list(range(NCORES)))
    return _unshard(res.results)


def kernel_profiled(x: np.ndarray):
    """Like kernel() but with NTFF tracing; returns (out, BassKernelResults)."""
    nc = _get_nc()
    res = run_bass_kernel_spmd(
        nc, _shard(x), core_ids=list(range(NCORES)), trace=True
    )
    return _unshard(res.results), res


# revision 3
# speedup vs baseline: 1.1959x; 1.1959x over previous
"""Trainium2 Bass kernel for nn_BatchDelayProcessor.

Computes, per batch row (B=64, T=441000, D=22050 delay, 20 blocks):
    delayed[t] = 0                          , t < D
    delayed[t] = x[t-D] + 0.3*delayed[t-D]  , t >= D
    out[t]     = 0.5*x[t] + 0.5*delayed[t]

With u_k = 0.5*x_k (folded into the host-side bf16 conversion -- an exact
exponent shift), the block recurrence unrolls to a dense lower-triangular
combination:
    out_m = u_m + sum_{j<m} 0.3^(m-1-j) * u_j  =  sum_j A[m,j] u_j
so the whole kernel is ONE 20x20 matrix applied per sample position --
perfect for the otherwise-idle PE array, with NO serial chain anywhere.
Six independent position-groups are folded into a 120x120 block-diagonal
stationary, so each matmul tile computes 6 groups x 20 blocks at once.

Pipeline (per core): loads -> PE (60 tiles of 490 cols) -> PSUM drain
(split DVE even tiles / ACT odd tiles, f32->bf16 downcast) -> stores.
Everything is column-chunked and fully overlapped.

Perf notes vs the 128us f32 baseline:
  - bf16 I/O halves HBM traffic to 7.06 MB each way (tol 2e-2, measured
    err ~5e-3); floor = 14.1MB / 358GB/s ~= 39us.
  - Host re-layout to (120 partitions x 29400) gives per-partition
    DRAM-contiguous runs: 120 big descriptors per DMA.
  - Bulk DMA via SWDGE (gpsimd) spreads over all 16 SDMA engines (HWDGE
    only ever lands on engines 0-7); the first two load groups + the
    stationary go via sync/HWDGE because the Q7 SWDGE path takes ~7us to
    boot while HWDGE starts at ~2.5us.
  - v1 (DVE scalar_tensor_tensor chain) measured 85.6us: STT supports no
    DVE perf modes (1 elem/cycle) and the 61us DVE chain was critical.
    The PE formulation moves the math to an idle engine at ~0.2us/tile.

Sharding: data-parallel over batch -- 8 rows per NeuronCore, no comms.
Layout: partition p = g*20 + k holds block k of position-group g; its
29400 columns are sample positions (flattened (row, d) // 6 groups).
"""

from contextlib import ExitStack

import ml_dtypes
import numpy as np

import concourse.bass as bass
import concourse.mybir as mybir
from concourse.bass_utils import run_bass_kernel_spmd

B, T = 64, 441000
D, NBLK = 22050, 20
NCORES = 8
ROWS = B // NCORES          # 8 rows per core
NG = 6                      # position-groups folded into the stationary
P = NG * NBLK               # 120 partitions
PW = ROWS * D // NG         # 29400 positions per group-lane

TN = 490                    # moving-tile columns (1960 B psum = 1 bank)
NT = PW // TN               # 60 tiles
NPB = 8                     # psum bank ring

GL = 2940                   # columns per load DMA (6 tiles)
NGL = PW // GL              # 10 load groups
NHW = 2                     # first NHW load groups go via sync/HWDGE
GS = 2940                   # columns per store DMA
NGS = PW // GS              # 10 store groups
TPG = GL // TN              # 6 tiles per load group
FEEDBACK, MIX = 0.3, 0.5

BF16 = mybir.dt.bfloat16
F32 = mybir.dt.float32
BF16_NP = ml_dtypes.bfloat16


def _amat() -> np.ndarray:
    """Block-diag stationary lhsT[(g,k),(g',m)] = A[m,k] * (g==g')."""
    A = np.zeros((NBLK, NBLK), dtype=np.float64)
    for m in range(NBLK):
        A[m, m] = 1.0
        for j in range(m):
            A[m, j] = FEEDBACK ** (m - 1 - j)
    lhsT = np.zeros((P, P), dtype=np.float64)
    for g in range(NG):
        lhsT[g * NBLK : (g + 1) * NBLK, g * NBLK : (g + 1) * NBLK] = A.T
    return lhsT.astype(BF16_NP)


def build_nc() -> bass.Bass:
    nc = bass.Bass(trn_type="TRN2")
    a = nc.declare_dram_parameter("a", [P, P], BF16, isOutput=False)
    x = nc.declare_dram_parameter("x", [P, PW], BF16, isOutput=False)
    y = nc.declare_dram_parameter("y", [P, PW], BF16, isOutput=True)
    xv = x.rearrange("p (g f) -> g p f", g=NGL)
    yv = y.rearrange("p (s f) -> s p f", s=NGS)

    with ExitStack() as ctx:
        block = ctx.enter_context(nc.Block())
        abuf = ctx.enter_context(nc.sbuf_tensor("abuf", [P, P], BF16))
        xbuf = ctx.enter_context(nc.sbuf_tensor("xbuf", [P, PW], BF16))
        obuf = ctx.enter_context(nc.sbuf_tensor("obuf", [P, PW], BF16))
        pb = [
            ctx.enter_context(nc.psum_tensor(f"pb{j}", [P, TN], F32))
            for j in range(NPB)
        ]
        s_a = ctx.enter_context(nc.semaphore("s_a"))
        s_in = [
            ctx.enter_context(nc.semaphore(f"s_in{g}")) for g in range(NGL)
        ]
        s_pe = ctx.enter_context(nc.semaphore("s_pe"))
        s_dve = ctx.enter_context(nc.semaphore("s_dve"))
        s_act = ctx.enter_context(nc.semaphore("s_act"))
        s_st = ctx.enter_context(nc.semaphore("s_st"))

        def cols(buf, c0, n):
            return buf[:, c0 : c0 + n]

        # Early loads on the HWDGE path (sync sequencer): the stationary
        # plus the first two column groups.  HWDGE starts ~2.5us into the
        # kernel while the gpsimd/SWDGE path takes ~7us to emit its first
        # descriptor; HWDGE's engine 0-7 restriction doesn't matter while
        # HBM is otherwise idle.
        @block.sync
        def _(sync):
            sync.dma_start(out=abuf[:, :], in_=a[:, :]).then_inc(s_a, 16)
            for g in range(NHW):
                sync.dma_start(
                    out=cols(xbuf, g * GL, GL), in_=xv[g]
                ).then_inc(s_in[g], 16)

        # Bulk loads + all stores on SWDGE (all 16 SDMA engines).
        @block.gpsimd
        def _(gpsimd):
            for g in range(NHW, NGL):
                gpsimd.dma_start(
                    out=cols(xbuf, g * GL, GL), in_=xv[g]
                ).then_inc(s_in[g], 16)
            for s in range(NGS):
                # store group s needs tiles [6s, 6s+6) drained: by then DVE
                # (even tiles) and ACT (odd tiles) each finished 3s+3.
                gpsimd.wait_ge(s_dve, 3 * s + 3)
                gpsimd.wait_ge(s_act, 3 * s + 3)
                gpsimd.dma_start(
                    out=yv[s], in_=cols(obuf, s * GS, GS)
                ).then_inc(s_st, 16)

        # PE: one self-loading matmul per 490-column tile against the
        # block-diagonal stationary; psum banks cycle mod 8.
        @block.tensor
        def _(tensor):
            tensor.wait_ge(s_a, 16)
            for t in range(NT):
                if t % TPG == 0:
                    tensor.wait_ge(s_in[t // TPG], 16)
                if t >= NPB:
                    # bank WAR: tile t-8's drain must have read the bank
                    td = t - NPB
                    if td % 2 == 0:
                        tensor.wait_ge(s_dve, td // 2 + 1)
                    else:
                        tensor.wait_ge(s_act, td // 2 + 1)
                nc.tensor.matmul(
                    out=pb[t % NPB][:, :],
                    lhsT=abuf[:, :],
                    rhs=cols(xbuf, t * TN, TN),
                    start=True,
                    stop=True,
                ).then_inc(s_pe, 1)

        # PSUM drain, split across DVE (even tiles) and ACT (odd tiles);
        # both do the f32 -> bf16 downcast into obuf.
        @block.vector
        def _(vector):
            for t in range(0, NT, 2):
                vector.wait_ge(s_pe, t + 1)
                nc.vector.tensor_copy(
                    cols(obuf, t * TN, TN), pb[t % NPB][:, :]
                ).then_inc(s_dve, 1)

        @block.scalar
        def _(scalar):
            for t in range(1, NT, 2):
                scalar.wait_ge(s_pe, t + 1)
                nc.scalar.copy(
                    cols(obuf, t * TN, TN), pb[t % NPB][:, :]
                ).then_inc(s_act, 1)

    return nc


_NC_CACHE = None


def _get_nc() -> bass.Bass:
    global _NC_CACHE
    if _NC_CACHE is None:
        _NC_CACHE = build_nc()
    return _NC_CACHE


_A_BF16 = _amat()


def _shard(x: np.ndarray) -> list[dict[str, np.ndarray]]:
    x = np.asarray(x, dtype=np.float32)
    assert x.shape == (B, T), x.shape
    maps = []
    for i in range(NCORES):
        u = (x[i * ROWS : (i + 1) * ROWS] * np.float32(MIX)).reshape(
            ROWS, NBLK, D
        )
        # (r, k, d) -> (k, r*d) -> (k, g, j) -> (g, k, j) -> (120, 29400)
        u = u.transpose(1, 0, 2).reshape(NBLK, NG, PW).transpose(1, 0, 2)
        maps.append(
            {
                "a": _A_BF16,
                "x": np.ascontiguousarray(u).reshape(P, PW).astype(BF16_NP),
            }
        )
    return maps


def _unshard(results: list[dict[str, np.ndarray]]) -> np.ndarray:
    outs = []
    for r in results:
        yc = np.asarray(r["y"]).astype(np.float32).reshape(NG, NBLK, PW)
        yc = yc.transpose(1, 0, 2).reshape(NBLK, ROWS, D).transpose(1, 0, 2)
        outs.append(np.ascontiguousarray(yc).reshape(ROWS, T))
    return np.concatenate(outs, axis=0)


def kernel(x: np.ndarray) -> np.ndarray:
    nc = _get_nc()
    res = run_bass_kernel_spmd(nc, _shard(x), core_ids=list(range(NCORES)))
    return _unshard(res.results)


def kernel_profiled(x: np.ndarray):
    """Like kernel() but with NTFF tracing; returns (out, BassKernelResults)."""
    nc = _get_nc()
    res = run_bass_kernel_spmd(
        nc, _shard(x), core_ids=list(range(NCORES)), trace=True
    )
    return _unshard(res.results), res


# revision 4
# speedup vs baseline: 1.5345x; 1.2832x over previous
"""Trainium2 Bass kernel for nn_BatchDelayProcessor.

Computes, per batch row (B=64, T=441000, D=22050 delay, 20 blocks):
    delayed[t] = 0                          , t < D
    delayed[t] = x[t-D] + 0.3*delayed[t-D]  , t >= D
    out[t]     = 0.5*x[t] + 0.5*delayed[t]

With u_k = 0.5*x_k (folded into the host-side bf16 conversion -- an exact
exponent shift), the block recurrence unrolls to a dense lower-triangular
combination:
    out_m = u_m + sum_{j<m} 0.3^(m-1-j) * u_j  =  sum_j A[m,j] u_j
so the whole kernel is ONE 20x20 matrix applied per sample position --
perfect for the otherwise-idle PE array, with NO serial chain anywhere.
Six independent position-groups are folded into a 120x120 block-diagonal
stationary, so each matmul tile computes 6 groups x 20 blocks at once.

Pipeline (per core): loads -> PE (60 tiles of 490 cols) -> PSUM drain
(split DVE even tiles / ACT odd tiles, f32->bf16 downcast) -> stores.

Schedule notes (evidence from prior iterations' perfetto traces):
  - bf16 I/O halves HBM traffic to 7.06 MB each way (tol 2e-2, measured
    err ~4e-3); HBM floor = 14.1MB / 358GB/s ~= 39us + ~7us fixed
    preamble + ~3us end fences.
  - All x loads go through SWDGE (gpsimd) IN CONSUMPTION ORDER with
    telescoping sizes (small first so the PE starts ~2us after first
    bytes).  v2 put the first groups on a second queue, and the SDMA
    engines' packet round-robin between queues starved them -- the PE
    sat idle until 20us.
  - All stores go on the sync/HWDGE ring: a separate queue that the
    engines round-robin against the SWDGE load queue, so stores stream
    during loads instead of queueing FIFO behind them (v2 lost ~16us to
    a store backlog after compute finished).  HWDGE descriptor
    generation runs on the issuing sequencer (~1.5us per DMA), which is
    why stores are NOT issued from the scalar sequencer -- that would
    stall the ACT drain loop and the psum bank ring behind it.
  - Store descriptors are 11.8 KB (2-block column groups): store-side
    per-descriptor cost dominated v2's tail at 5.9 KB.

Sharding: data-parallel over batch -- 8 rows per NeuronCore, no comms.
Layout: partition p = g*20 + k holds block k of position-group g; its
29400 columns are sample positions (flattened (row, d), split into 6
contiguous groups).
"""

from contextlib import ExitStack

import ml_dtypes
import numpy as np

import concourse.bass as bass
import concourse.mybir as mybir
from concourse.bass_utils import run_bass_kernel_spmd

B, T = 64, 441000
D, NBLK = 22050, 20
NCORES = 8
ROWS = B // NCORES          # 8 rows per core
NG = 6                      # position-groups folded into the stationary
P = NG * NBLK               # 120 partitions
PW = ROWS * D // NG         # 29400 positions per group-lane

TN = 490                    # moving-tile columns (1960 B psum = 1 bank)
NT = PW // TN               # 60 tiles
NPB = 8                     # psum bank ring

# telescoping load groups (units: tiles); small first for fast PE start
LOAD_TILES = [3, 3, 6, 12, 12, 12, 12]
# store groups (tiles), all on the sync HWDGE ring, in order
STORE_TILES = [12, 12, 12, 12, 6, 6]
FEEDBACK, MIX = 0.3, 0.5

BF16 = mybir.dt.bfloat16
F32 = mybir.dt.float32
BF16_NP = ml_dtypes.bfloat16


def _amat() -> np.ndarray:
    """Block-diag stationary lhsT[(g,k),(g',m)] = A[m,k] * (g==g')."""
    A = np.zeros((NBLK, NBLK), dtype=np.float64)
    for m in range(NBLK):
        A[m, m] = 1.0
        for j in range(m):
            A[m, j] = FEEDBACK ** (m - 1 - j)
    lhsT = np.zeros((P, P), dtype=np.float64)
    for g in range(NG):
        lhsT[g * NBLK : (g + 1) * NBLK, g * NBLK : (g + 1) * NBLK] = A.T
    return lhsT.astype(BF16_NP)


def build_nc() -> bass.Bass:
    assert sum(LOAD_TILES) == NT and sum(STORE_TILES) == NT
    load_hi = np.cumsum(LOAD_TILES).tolist()      # exclusive upper tile
    store_lo = [0] + np.cumsum(STORE_TILES).tolist()[:-1]

    nc = bass.Bass(trn_type="TRN2")
    a = nc.declare_dram_parameter("a", [P, P], BF16, isOutput=False)
    x = nc.declare_dram_parameter("x", [P, PW], BF16, isOutput=False)
    y = nc.declare_dram_parameter("y", [P, PW], BF16, isOutput=True)

    with ExitStack() as ctx:
        block = ctx.enter_context(nc.Block())
        abuf = ctx.enter_context(nc.sbuf_tensor("abuf", [P, P], BF16))
        xbuf = ctx.enter_context(nc.sbuf_tensor("xbuf", [P, PW], BF16))
        obuf = ctx.enter_context(nc.sbuf_tensor("obuf", [P, PW], BF16))
        pb = [
            ctx.enter_context(nc.psum_tensor(f"pb{j}", [P, TN], F32))
            for j in range(NPB)
        ]
        s_a = ctx.enter_context(nc.semaphore("s_a"))
        s_in = [
            ctx.enter_context(nc.semaphore(f"s_in{g}"))
            for g in range(len(LOAD_TILES))
        ]
        s_pe = ctx.enter_context(nc.semaphore("s_pe"))
        s_dve = ctx.enter_context(nc.semaphore("s_dve"))
        s_act = ctx.enter_context(nc.semaphore("s_act"))
        s_st = ctx.enter_context(nc.semaphore("s_st"))

        def tcols(buf, t0, t1):
            return buf[:, t0 * TN : t1 * TN]

        # SWDGE: stationary first (tiny), then x loads in consumption
        # order.  No waits anywhere -- descriptor-ring backpressure is
        # the only limiter, engines drain this queue FIFO.
        @block.gpsimd
        def _(gpsimd):
            gpsimd.dma_start(out=abuf[:, :], in_=a[:, :]).then_inc(s_a, 16)
            t0 = 0
            for g, t1 in enumerate(load_hi):
                gpsimd.dma_start(
                    out=tcols(xbuf, t0, t1),
                    in_=x[:, t0 * TN : t1 * TN],
                ).then_inc(s_in[g], 16)
                t0 = t1

        # Stores on the sync HWDGE ring (idle sequencer, separate DMA
        # queue -> engines interleave stores with SWDGE loads at packet
        # granularity instead of FIFO-queueing them behind all loads).
        @block.sync
        def _(sync):
            for s, st0 in enumerate(store_lo):
                st1 = st0 + STORE_TILES[s]
                sync.wait_ge(s_dve, st1 // 2)
                sync.wait_ge(s_act, st1 // 2)
                sync.dma_start(
                    out=y[:, st0 * TN : st1 * TN],
                    in_=tcols(obuf, st0, st1),
                ).then_inc(s_st, 16)

        # PE: one self-loading matmul per 490-column tile against the
        # block-diagonal stationary; psum banks cycle mod 8.
        @block.tensor
        def _(tensor):
            tensor.wait_ge(s_a, 16)
            g = -1
            for t in range(NT):
                gt = next(i for i, hi in enumerate(load_hi) if t < hi)
                if gt > g:
                    tensor.wait_ge(s_in[gt], 16)
                    g = gt
                if t >= NPB:
                    td = t - NPB  # bank WAR: tile td's drain read the bank
                    if td % 2 == 0:
                        tensor.wait_ge(s_dve, td // 2 + 1)
                    else:
                        tensor.wait_ge(s_act, td // 2 + 1)
                nc.tensor.matmul(
                    out=pb[t % NPB][:, :],
                    lhsT=abuf[:, :],
                    rhs=tcols(xbuf, t, t + 1),
                    start=True,
                    stop=True,
                ).then_inc(s_pe, 1)

        # PSUM drain, split across DVE (even tiles) and ACT (odd tiles);
        # both do the f32 -> bf16 downcast into obuf.
        @block.vector
        def _(vector):
            for t in range(0, NT, 2):
                vector.wait_ge(s_pe, t + 1)
                nc.vector.tensor_copy(
                    tcols(obuf, t, t + 1), pb[t % NPB][:, :]
                ).then_inc(s_dve, 1)

        @block.scalar
        def _(scalar):
            for t in range(1, NT, 2):
                scalar.wait_ge(s_pe, t + 1)
                nc.scalar.copy(
                    tcols(obuf, t, t + 1), pb[t % NPB][:, :]
                ).then_inc(s_act, 1)

    return nc


_NC_CACHE = None


def _get_nc() -> bass.Bass:
    global _NC_CACHE
    if _NC_CACHE is None:
        _NC_CACHE = build_nc()
    return _NC_CACHE


_A_BF16 = _amat()


def _shard(x: np.ndarray) -> list[dict[str, np.ndarray]]:
    x = np.asarray(x, dtype=np.float32)
    assert x.shape == (B, T), x.shape
    maps = []
    for i in range(NCORES):
        u = (x[i * ROWS : (i + 1) * ROWS] * np.float32(MIX)).reshape(
            ROWS, NBLK, D
        )
        # (r, k, d) -> (k, r*d) -> (k, g, j) -> (g, k, j) -> (120, 29400)
        u = u.transpose(1, 0, 2).reshape(NBLK, NG, PW).transpose(1, 0, 2)
        maps.append(
            {
                "a": _A_BF16,
                "x": np.ascontiguousarray(u).reshape(P, PW).astype(BF16_NP),
            }
        )
    return maps


def _unshard(results: list[dict[str, np.ndarray]]) -> np.ndarray:
    outs = []
    for r in results:
        yc = np.asarray(r["y"]).astype(np.float32).reshape(NG, NBLK, PW)
        yc = yc.transpose(1, 0, 2).reshape(NBLK, ROWS, D).transpose(1, 0, 2)
        outs.append(np.ascontiguousarray(yc).reshape(ROWS, T))
    return np.concatenate(outs, axis=0)


def kernel(x: np.ndarray) -> np.ndarray:
    nc = _get_nc()
    res = run_bass_kernel_spmd(nc, _shard(x), core_ids=list(range(NCORES)))
    return _unshard(res.results)


def kernel_profiled(x: np.ndarray):
    """Like kernel() but with NTFF tracing; returns (out, BassKernelResults)."""
    nc = _get_nc()
    res = run_bass_kernel_spmd(
        nc, _shard(x), core_ids=list(range(NCORES)), trace=True
    )
    return _unshard(res.results), res
